# revision 18
# baseline (speedup 1.0000x reference)
"""Trainium2 Bass kernel for nn_CombineMultiOutputModelWeightedConcat.

Strategy: pure data-parallel over 8 NeuronCores (batch 32768 -> 4096/core).
Activations are kept feature-major ([feat(P), batch(free)]) so every layer's
matmul contracts over SBUF partitions and BatchNorm stats are per-partition
reductions along the free axis (bn_stats/bn_aggr on DVE).

BatchNorm (training-mode, full-batch stats) is handled by:
  1. computing per-core per-feature (mean, E[x^2]) with bn_stats/bn_aggr,
  2. a tiny AllReduce across the 8 cores,
  3. folding the resulting affine (s = g*rsqrt(var+eps), t = be - mean*s)
     into the *next* layer's weights (W' = diag(s) W) and bias (b' = t@W + b),
so no per-element normalization pass ever touches the activations.

Matmuls run in float32r (full PE rate at fp32 storage precision ~1e-4).
The branch tensors b1x/b2x ([4,B,256] per stream) don't fit in SBUF in fp32;
they are spilled to HBM at production and streamed back for the gating and
combine stages.

The 2-way softmax gate y = softmax(z) is computed as y0 = sigmoid(z0-z1),
y1 = sigmoid(z1-z0) via a single folded difference column wy_diff.
The combine layer xw @ Wc with xw = [bn(b1x)*y0, bn(b2x)*y1] uses
   xc_pre = y0*(b1x@Wc_top' + t1@Wc_top) + y1*(b2x@Wc_bot' + t2@Wc_bot) + bc
where the t@Wc rows are added into PSUM with K=1 ones-row matmuls and the
per-sample y scaling happens on DVE against gpsimd-partition-broadcast rows.
"""

import numpy as np

import concourse.bass as bass
import concourse.mybir as mybir
import concourse.tile as tile
from concourse import bacc, bass_utils

P = 128
T = 512                      # batch tile (one PSUM bank of fp32)
N_CORES = 8
B_FULL = 32768
BL = B_FULL // N_CORES       # 4096 per-core batch
NT = BL // T                 # 8 tiles
EPS = 0.2

f32 = mybir.dt.float32
f32r = mybir.dt.float32r
AF = mybir.ActivationFunctionType
ALU = mybir.AluOpType

HEAD_SIZES = [3, 3, 6, 4]


def build(n_cores=N_CORES, mock_collectives=False):
    nc = bacc.Bacc("TRN2", target_bir_lowering=False, debug=False,
                   num_devices=n_cores)
    RG = [list(range(n_cores))]

    def din(name, shape, dt=f32r):
        return nc.dram_tensor(name, shape, dt, kind="ExternalInput")

    X = [din("x1t", [512, BL]), din("x2t", [512, BL])]
    W1 = din("W1", [512, 256]);  b1 = din("b1", [256, 1], f32)
    W2 = din("W2", [256, 256]);  b2 = din("b2", [256, 1], f32)
    Wbx = [din("Wbx1", [4, 256, 256]), din("Wbx2", [4, 256, 256])]
    bbx = [din("bbx1", [4, 256, 1], f32), din("bbx2", [4, 256, 1], f32)]
    Wwa = din("Wwa", [4, 512, 128]); bwa = din("bwa", [4, 128, 1], f32)
    Wwb = din("Wwb", [4, 128, 64]);  bwb = din("bwb", [4, 64, 1], f32)
    Wwc = din("Wwc", [4, 64, 64]);   bwc = din("bwc", [4, 64, 1], f32)
    Wyd = din("Wyd", [4, 64, 1]);    byd = din("byd", [4, 1, 1], f32)
    Wc = din("Wc", [4, 512, 256]);   bc = din("bc", [4, 256, 1], f32)
    Wo = [din(f"Wo{k+1}", [256, HEAD_SIZES[k]]) for k in range(4)]
    bo = [din(f"bo{k+1}", [HEAD_SIZES[k], 1], f32) for k in range(4)]
    G = {n: din(n, [d, 1], f32) for n, d in
         [("g1", 256), ("be1", 256), ("g2", 256), ("be2", 256),
          ("g3", 128), ("be3", 128), ("g4", 64), ("be4", 64)]}
    O = [nc.dram_tensor(f"o{k+1}", [HEAD_SIZES[k], BL], f32,
                        kind="ExternalOutput") for k in range(4)]

    with tile.TileContext(nc) as tc:
        from contextlib import ExitStack
        _stk = ExitStack()
        wp = _stk.enter_context(tc.tile_pool(name="wp", bufs=1))
        wtr = _stk.enter_context(tc.tile_pool(name="wtr", bufs=4))
        tcp = _stk.enter_context(tc.tile_pool(name="tcp", bufs=4))
        act = _stk.enter_context(tc.tile_pool(name="act", bufs=7))
        wbig = _stk.enter_context(tc.tile_pool(name="wbig", bufs=8))
        xin = _stk.enter_context(tc.tile_pool(name="xin", bufs=4))
        zst = _stk.enter_context(tc.tile_pool(name="zst", bufs=4))
        zin = _stk.enter_context(tc.tile_pool(name="zin", bufs=5))
        ybp = _stk.enter_context(tc.tile_pool(name="ybp", bufs=2))
        htm = _stk.enter_context(tc.tile_pool(name="htm", bufs=2))
        ysp = _stk.enter_context(tc.tile_pool(name="ysp", bufs=2))
        psmm = _stk.enter_context(tc.tile_pool(name="psmm", bufs=6, space="PSUM"))
        pss = _stk.enter_context(tc.tile_pool(name="pss", bufs=2, space="PSUM"))
        drp = _stk.enter_context(tc.tile_pool(name="drp", bufs=1, space="DRAM"))

        # ---------- weight / bias / gamma loads ----------
        def load_chunks(name, src, kchunks, width, pool=wp, part=P, tag=None):
            out = []
            for k in range(kchunks):
                t_ = pool.tile([part, width], f32r, name=f"{name}_{k}",
                               tag=(tag or f"{name}_{k}"))
                nc.sync.dma_start(t_[:], src[k * part:(k + 1) * part, :])
                out.append(t_)
            return out

        W1t = load_chunks("W1t", W1.ap(), 4, 256, pool=wtr, tag="wtrunk")
        W2t = load_chunks("W2t", W2.ap(), 2, 256)
        # per-stream folded copies of W2 (trunk L2)
        W2f = [[wtr.tile([P, 256], f32r, name=f"W2f_{s}_{k}", tag="wtrunk")
                for k in range(2)] for s in range(2)]
        Wbxt = [[None] * 4, [None] * 4]
        Wwat = [load_chunks(f"Wwa_{a}", Wwa.ap()[a], 4, 128) for a in range(4)]
        Wwbt = [load_chunks(f"Wwb_{a}", Wwb.ap()[a], 1, 64) for a in range(4)]
        Wwct = [load_chunks(f"Wwc_{a}", Wwc.ap()[a], 1, 64, part=64)
                for a in range(4)]
        Wydt = [load_chunks(f"Wyd_{a}", Wyd.ap()[a], 1, 1, part=64)
                for a in range(4)]
        Wct = [None] * 4
        Wot = [load_chunks(f"Wo_{k}", Wo[k].ap(), 2, HEAD_SIZES[k])
               for k in range(4)]

        def load_col(name, src, part=P):
            t_ = wp.tile([part, 1], f32, name=name)
            nc.sync.dma_start(t_[:], src)
            return t_

        b1t = [load_col(f"b1t{m}", b1.ap()[m * P:(m + 1) * P]) for m in range(2)]
        b2t = [load_col(f"b2t{m}", b2.ap()[m * P:(m + 1) * P]) for m in range(2)]
        bbxt = [[[load_col(f"bbx{s}_{a}_{m}", bbx[s].ap()[a, m * P:(m + 1) * P])
                  for m in range(2)] for a in range(4)] for s in range(2)]
        bwat = [load_col(f"bwa{a}", bwa.ap()[a]) for a in range(4)]
        bwbt = [load_col(f"bwb{a}", bwb.ap()[a], part=64) for a in range(4)]
        bwct = [load_col(f"bwc{a}", bwc.ap()[a], part=64) for a in range(4)]
        bydt = [load_col(f"byd{a}", byd.ap()[a], part=1) for a in range(4)]
        bct = [[load_col(f"bct{a}_{m}", bc.ap()[a, m * P:(m + 1) * P])
                for m in range(2)] for a in range(4)]
        bot = [load_col(f"bot{k}", bo[k].ap(), part=HEAD_SIZES[k])
               for k in range(4)]
        Gt = {}
        for n, d in [("g1", 256), ("be1", 256), ("g2", 256), ("be2", 256)]:
            Gt[n] = [load_col(f"{n}_{m}", G[n].ap()[m * P:(m + 1) * P])
                     for m in range(2)]
        for n in ("g3", "be3"):
            Gt[n] = [load_col(f"{n}_0", G[n].ap())]
        for n in ("g4", "be4"):
            Gt[n] = [load_col(f"{n}_0", G[n].ap(), part=64)]

        ones_tmp = ysp.tile([1, T], f32, tag="ys", name="ones_tmp")
        nc.vector.memset(ones_tmp[:], 1.0)
        onesrow = wp.tile([1, T], f32r, name="onesrow")
        nc.vector.tensor_copy(onesrow[:], ones_tmp[:])
        onec_tmp = wp.tile([6, 1], f32, name="onec_tmp")
        nc.vector.memset(onec_tmp[:], 1.0)
        onescol6 = wp.tile([6, 1], f32r, name="onescol6")
        nc.vector.tensor_copy(onescol6[:], onec_tmp[:])
        onescol = {n: onescol6[:n] for n in set(HEAD_SIZES)}

        # zb spill space in HBM: [stream, attr, mchunk, P, BL]
        zbd = drp.tile([2, 4, 2, P, BL], f32r, name="zbd")

        # ---------- generic helpers ----------
        def mm_layer(name, in_chunks, w_at, bias_at, out_chunks, st6s,
                     mpart=P):
            """out[m][:, t] = relu(sum_k w(m,k).T @ in[k][:,t] + bias(m));
            bn_stats into st6s[m][:, t, :]."""
            nm = len(out_chunks)
            nk = len(in_chunks)
            for t in range(NT):
                sl = slice(t * T, (t + 1) * T)
                for m in range(nm):
                    pt = psmm.tile([P, T], f32, tag="mm",
                                   name=f"{name}ps{t}_{m}")
                    for j in range(nk):
                        nc.tensor.matmul(pt[:mpart], w_at(m, j),
                                         in_chunks[j][:, sl],
                                         start=(j == 0), stop=(j == nk - 1))
                    nc.scalar.activation(out_chunks[m][:, sl], pt[:mpart],
                                         AF.Relu, bias=bias_at(m))
                    nc.vector.bn_stats(st6s[m][:, t, :], out_chunks[m][:, sl])

        def new_st6(name, n, part=P):
            return [wp.tile([part, NT, 6], f32, name=f"{name}_st6_{i}")
                    for i in range(n)]

        ar_ctr = [0]

        def stats_to_affine(name, st6s, gammas, betas, part=P):
            """bn_aggr each chunk, AllReduce (mean/NC, E[x^2]/NC) across
            cores, return [(s, t), ...] per chunk as f32r [part,1] slices.
            All finalize arithmetic is batched over chunks."""
            c = len(st6s)
            mva = wp.tile([part, 2, c], f32, name=f"{name}_mva")
            for i, st6 in enumerate(st6s):
                nc.vector.bn_aggr(mva[:, :, i], st6[:])
            mean = mva[:, 0, :]
            var = mva[:, 1, :]
            tmp = wp.tile([part, c], f32, name=f"{name}_tmp")
            nc.vector.tensor_tensor(tmp[:], mean, mean, ALU.mult)
            nc.vector.tensor_tensor(tmp[:], var, tmp[:], ALU.add)
            pack = wp.tile([part, 2, c], f32, name=f"{name}_pack")
            nc.vector.tensor_scalar_mul(pack[:, 0, :], mean, 1.0 / n_cores)
            nc.vector.tensor_scalar_mul(pack[:, 1, :], tmp[:], 1.0 / n_cores)
            ar_ctr[0] += 1
            cin = drp.tile([part, 2 * c], f32, name=f"{name}_cin")
            cout = drp.tile([part, 2 * c], f32, name=f"{name}_cout")
            nc.sync.dma_start(cin[:], pack[:].rearrange("p a b -> p (a b)"))
            if mock_collectives:
                nc.sync.dma_start(cout[:], cin[:])
            else:
                nc.gpsimd.collective_compute(
                    "AllReduce", ALU.add, replica_groups=RG,
                    ins=[cin.opt()], outs=[cout.opt()])
            res = wp.tile([part, 2, c], f32, name=f"{name}_res")
            nc.sync.dma_start(res[:].rearrange("p a b -> p (a b)"), cout[:])
            gm = res[:, 0, :]
            gq = res[:, 1, :]
            x = wp.tile([part, c], f32, name=f"{name}_x")
            nc.vector.tensor_tensor(x[:], gm, gm, ALU.mult)
            nc.vector.tensor_tensor(x[:], gq, x[:], ALU.subtract)
            nc.vector.tensor_scalar_add(x[:], x[:], EPS)
            iv = wp.tile([part, c], f32, name=f"{name}_iv")
            nc.vector.reciprocal(iv[:], x[:])
            rs = wp.tile([part, c], f32, name=f"{name}_rs")
            nc.scalar.activation(rs[:], iv[:], AF.Sqrt)
            u = wp.tile([part, c], f32, name=f"{name}_u")
            for _ in range(2):   # Newton-refine rsqrt
                nc.vector.tensor_tensor(u[:], x[:], rs[:], ALU.mult)
                nc.vector.tensor_tensor(u[:], u[:], rs[:], ALU.mult)
                nc.vector.tensor_scalar(u[:], u[:], -0.5, 1.5,
                                        ALU.mult, ALU.add)
                nc.vector.tensor_tensor(rs[:], rs[:], u[:], ALU.mult)
            gpk = wp.tile([part, c], f32, name=f"{name}_gpk")
            bpk = wp.tile([part, c], f32, name=f"{name}_bpk")
            for i in range(c):
                nc.vector.tensor_copy(gpk[:, i:i + 1], gammas[i][:])
                nc.vector.tensor_copy(bpk[:, i:i + 1], betas[i][:])
            sall = wp.tile([part, c], f32r, name=f"{name}_sall")
            nc.vector.tensor_tensor(sall[:], gpk[:], rs[:], ALU.mult)
            tall = wp.tile([part, c], f32r, name=f"{name}_tall")
            nc.vector.tensor_tensor(tall[:], gm, sall[:], ALU.mult)
            nc.vector.tensor_tensor(tall[:], bpk[:], tall[:], ALU.subtract)
            return [(sall[:, i:i + 1], tall[:, i:i + 1]) for i in range(c)]

        def fold_bias_col(name, w_at, stf, b_orig, nm, nk, mpart=P):
            """b''[m] = sum_k w(m,k).T @ t[k] + b_orig[m] -> [mpart,1] f32."""
            outs = []
            for m in range(nm):
                pb = pss.tile([P, 1], f32, tag="pss", name=f"{name}_pb{m}")
                for j in range(nk):
                    # N=1 moving dim is illegal for fp32r -> plain fp32 matmul
                    nc.tensor.matmul(pb[:mpart], w_at(m, j).bitcast(f32),
                                     stf[j][1][:].bitcast(f32),
                                     start=(j == 0), stop=(j == nk - 1))
                bpp = wp.tile([mpart, 1], f32, name=f"{name}_b{m}")
                nc.vector.tensor_tensor(bpp[:], pb[:mpart], b_orig[m][:],
                                        ALU.add)
                outs.append(bpp)
            return outs

        def fold_weights(w_chunks, stf):
            """w[k] *= s[k] per input-feature row, in place (after any
            bias-fold matmuls that read the original weights)."""
            for k, wt_ in enumerate(w_chunks):
                sh = wt_.shape
                nc.vector.tensor_tensor(
                    wt_[:], wt_[:], stf[k][0][:].to_broadcast(tuple(sh)),
                    ALU.mult)

        # ================= stage A: trunk L1 (both streams) =================
        z1 = [[act.tile([P, BL], f32r, tag="act", name=f"z1_{s}_{m}")
               for m in range(2)] for s in range(2)]
        st_z1 = [new_st6(f"z1s{s}", 2) for s in range(2)]
        for s in range(2):
            xr = X[s].ap().rearrange("(ko p) n -> ko p n", p=P)
            xtiles = {}
            for t in range(NT):
                sl = slice(t * T, (t + 1) * T)
                for k in range(4):
                    xt = xin.tile([P, T], f32r, tag="xin",
                                  name=f"x{s}_{t}_{k}")
                    nc.sync.dma_start(xt[:], xr[k, :, sl])
                    xtiles[k] = xt
                for m in range(2):
                    pt = psmm.tile([P, T], f32, tag="mm",
                                   name=f"Aps{s}_{t}_{m}")
                    for j in range(4):
                        nc.tensor.matmul(pt[:], W1t[j][:, m * P:(m + 1) * P],
                                         xtiles[j][:],
                                         start=(j == 0), stop=(j == 3))
                    nc.scalar.activation(z1[s][m][:, sl], pt[:], AF.Relu,
                                         bias=b1t[m][:])
                    nc.vector.bn_stats(st_z1[s][m][:, t, :], z1[s][m][:, sl])

        # ========== stage B: trunk L2 per stream (after z1 stats AR) =======
        z2 = [[act.tile([P, BL], f32r, tag="act", name=f"z2_{s}_{m}")
               for m in range(2)] for s in range(2)]
        st_z2 = [new_st6(f"z2s{s}", 2) for s in range(2)]
        for s in range(2):
            stf = stats_to_affine(f"arz1s{s}", st_z1[s], Gt["g1"], Gt["be1"])
            b2pp = fold_bias_col(f"fb2s{s}",
                                 lambda m, k: W2t[k][:, m * P:(m + 1) * P],
                                 stf, b2t, 2, 2)
            for k in range(2):   # fold into fresh per-stream copies
                nc.vector.tensor_tensor(
                    W2f[s][k][:], W2t[k][:],
                    stf[k][0][:].to_broadcast((P, 256)), ALU.mult)
            mm_layer(f"B{s}", [z1[s][0], z1[s][1]],
                     lambda m, k: W2f[s][k][:, m * P:(m + 1) * P],
                     lambda m: b2pp[m][:], [z2[s][0], z2[s][1]],
                     st_z2[s])

        # ===== stage C: branch projections per stream -> spill to HBM ======
        st_zb = [[new_st6(f"zb{s}_{a}", 2) for a in range(4)]
                 for s in range(2)]
        for s in range(2):
            for a in range(4):
                Wbxt[s][a] = load_chunks(f"Wbx{s}_{a}", Wbx[s].ap()[a], 2,
                                         256, pool=wbig, tag="wbig")
            stf = stats_to_affine(f"arz2s{s}", st_z2[s], Gt["g2"], Gt["be2"])
            for a in range(4):
                bxpp = fold_bias_col(
                    f"fbx{s}_{a}",
                    lambda m, k, a=a: Wbxt[s][a][k][:, m * P:(m + 1) * P],
                    stf, bbxt[s][a], 2, 2)
                fold_weights(Wbxt[s][a], stf)
                for t in range(NT):
                    sl = slice(t * T, (t + 1) * T)
                    zt = zst.tile([P, 2, T], f32r, tag="zst",
                                  name=f"zb{s}_{a}_{t}")
                    for m in range(2):
                        pt = psmm.tile([P, T], f32, tag="mm",
                                       name=f"Cps{s}_{a}_{t}_{m}")
                        for j in range(2):
                            nc.tensor.matmul(
                                pt[:], Wbxt[s][a][j][:, m * P:(m + 1) * P],
                                z2[s][j][:, sl],
                                start=(j == 0), stop=(j == 1))
                        nc.scalar.activation(zt[:, m, :], pt[:], AF.Relu,
                                             bias=bxpp[m][:])
                        nc.vector.bn_stats(st_zb[s][a][m][:, t, :],
                                           zt[:, m, :])
                    nc.sync.dma_start(
                        zbd[s, a][:, :, sl],
                        zt[:].rearrange("p m n -> m p n"))

        # zb affines per stream (4 attrs x 2 chunks each)
        stf_zb = []
        for s in range(2):
            flat = [st_zb[s][a][m] for a in range(4) for m in range(2)]
            gs = [Gt["g1"][m] for _ in range(4) for m in range(2)]
            bes = [Gt["be1"][m] for _ in range(4) for m in range(2)]
            stf_zb.append(stats_to_affine(f"arzb{s}", flat, gs, bes))
        # per attr: 4 k-chunks of xcat = [zb1_m0, zb1_m1, zb2_m0, zb2_m1]
        stf_cat = [[stf_zb[0][2 * a], stf_zb[0][2 * a + 1],
                    stf_zb[1][2 * a], stf_zb[1][2 * a + 1]] for a in range(4)]

        # ================= stage D: gating layer a ==========================
        za = [act.tile([P, BL], f32r, tag="act", name=f"za_{a}")
              for a in range(4)]
        st_za = new_st6("za", 4)
        for a in range(4):
            bapp = fold_bias_col(f"fwa{a}",
                                 lambda m, k, a=a: Wwat[a][k][:],
                                 stf_cat[a], [bwat[a]], 1, 4)
            fold_weights(Wwat[a], stf_cat[a])
            for t in range(NT):
                sl = slice(t * T, (t + 1) * T)
                zts = []
                for s in range(2):
                    zt = zin.tile([P, 2, T], f32r, tag="zin",
                                  name=f"Din{a}_{t}_{s}")
                    nc.sync.dma_start(zt[:],
                                      zbd[s, a][:, :, sl].rearrange(
                                          "m p n -> p m n"))
                    zts.append(zt)
                pt = psmm.tile([P, T], f32, tag="mm", name=f"Dps{a}_{t}")
                for j in range(4):
                    nc.tensor.matmul(pt[:], Wwat[a][j][:],
                                     zts[j // 2][:, j % 2, :],
                                     start=(j == 0), stop=(j == 3))
                nc.scalar.activation(za[a][:, sl], pt[:], AF.Relu,
                                     bias=bapp[0][:])
                nc.vector.bn_stats(st_za[a][:, t, :], za[a][:, sl])

        # ================= stage E: gating layer b_ =========================
        zbg = [act.tile([64, BL], f32r, tag="act", name=f"zbg_{a}")
               for a in range(4)]
        st_zbg = new_st6("zbg", 4, part=64)
        stf_za = stats_to_affine("arza", st_za, [Gt["g3"][0]] * 4,
                                 [Gt["be3"][0]] * 4)
        for a in range(4):
            bbpp = fold_bias_col(f"fwb{a}",
                                 lambda m, k, a=a: Wwbt[a][k][:],
                                 [stf_za[a]], [bwbt[a]], 1, 1, mpart=64)
            fold_weights(Wwbt[a], [stf_za[a]])
            mm_layer(f"E{a}", [za[a]], lambda m, k, a=a: Wwbt[a][k][:],
                     lambda m, a=a: bbpp[m][:], [zbg[a]],
                     [st_zbg[a]], mpart=64)

        # ================= stage F: gating layer c ==========================
        zcg = [act.tile([64, BL], f32r, tag="act", name=f"zcg_{a}")
               for a in range(4)]
        st_zcg = new_st6("zcg", 4, part=64)
        stf_zbg = stats_to_affine("arzbg", st_zbg, [Gt["g4"][0]] * 4,
                                  [Gt["be4"][0]] * 4, part=64)
        for a in range(4):
            bcpp = fold_bias_col(f"fwc{a}",
                                 lambda m, k, a=a: Wwct[a][k][:],
                                 [stf_zbg[a]], [bwct[a]], 1, 1, mpart=64)
            fold_weights(Wwct[a], [stf_zbg[a]])
            mm_layer(f"F{a}", [zbg[a]], lambda m, k, a=a: Wwct[a][k][:],
                     lambda m, a=a: bcpp[m][:], [zcg[a]],
                     [st_zcg[a]], mpart=64)

        # ============ stage G: gate y0/y1 via sigmoid of diff ==============
        dd = drp.tile([4, BL], f32, name="dd")
        stf_zcg = stats_to_affine("arzcg", st_zcg, [Gt["g4"][0]] * 4,
                                  [Gt["be4"][0]] * 4, part=64)
        for a in range(4):
            # cny = t_c @ wyd + byd  (scalar [1,1])
            pcy = pss.tile([1, 1], f32, tag="pss", name=f"Gc{a}")
            nc.tensor.matmul(pcy[:], stf_zcg[a][1][:].bitcast(f32),
                             Wydt[a][0][:].bitcast(f32),
                             start=True, stop=True)
            cny = wp.tile([1, 1], f32, name=f"cny{a}")
            nc.vector.tensor_tensor(cny[:], pcy[:], bydt[a][:], ALU.add)
            fold_weights(Wydt[a], [stf_zcg[a]])
            for t in range(NT):
                sl = slice(t * T, (t + 1) * T)
                pd = pss.tile([1, T], f32, tag="pss", name=f"Gd{a}_{t}")
                nc.tensor.matmul(pd[:], Wydt[a][0][:], zcg[a][:, sl],
                                 start=True, stop=True)
                ys0 = ysp.tile([1, T], f32, tag="ys", name=f"Gy0_{a}_{t}")
                nc.scalar.activation(ys0[:], pd[:], AF.Identity,
                                     bias=cny[:])
                nc.sync.dma_start(dd[a:a+1, sl], ys0[:])

        # ================= stage H: weighted combine =======================
        xcz = [act.tile([P, BL], f32r, tag="act", name=f"xcz_{i}")
               for i in range(8)]   # [attr*2 + m]
        st_xcz = new_st6("xcz", 8)
        tcw = [[None, None] for _ in range(4)]   # t@Wc rows per attr/half
        for a in range(4):
            Wct[a] = load_chunks(f"Wc_{a}", Wc.ap()[a], 4, 256, pool=wbig,
                                 tag="wbig")
            stf1 = [stf_zb[0][2 * a], stf_zb[0][2 * a + 1]]
            stf2 = [stf_zb[1][2 * a], stf_zb[1][2 * a + 1]]
            for h, stfh in ((0, stf1), (1, stf2)):
                pr = pss.tile([1, 256], f32, tag="pss", name=f"Hr{a}_{h}")
                for j in range(2):
                    nc.tensor.matmul(pr[:], stfh[j][1][:],
                                     Wct[a][2 * h + j][:],
                                     start=(j == 0), stop=(j == 1))
                row = tcp.tile([1, 256], f32r, tag="tcw", name=f"tcw{a}_{h}")
                nc.vector.tensor_copy(row[:], pr[:])
                tcw[a][h] = row
            fold_weights(Wct[a], stf1 + stf2)
            for t in range(NT):
                sl = slice(t * T, (t + 1) * T)
                zts = []
                for s in range(2):
                    zt = zin.tile([P, 2, T], f32r, tag="zin",
                                  name=f"Hin{a}_{t}_{s}")
                    nc.sync.dma_start(zt[:],
                                      zbd[s, a][:, :, sl].rearrange(
                                          "m p n -> p m n"))
                    zts.append(zt)
                ys0 = ysp.tile([1, T], f32, tag="ys", name=f"Hy0_{a}_{t}")
                nc.sync.dma_start(ys0[:], dd[a:a+1, sl])
                yb1 = ybp.tile([P, T], f32, tag="yb", name=f"yb1_{a}_{t}")
                nc.gpsimd.partition_broadcast(yb1[:], ys0[:])
                yb0 = ybp.tile([P, T], f32, tag="yb", name=f"yb0_{a}_{t}")
                nc.scalar.activation(yb0[:], yb1[:], AF.Sigmoid)
                nc.vector.tensor_scalar(yb1[:], yb0[:], -1.0, 1.0,
                                        ALU.mult, ALU.add)
                for m in range(2):
                    msl = slice(m * P, (m + 1) * P)
                    p1 = psmm.tile([P, T], f32, tag="mm",
                                   name=f"Hp1_{a}_{t}_{m}")
                    nc.tensor.matmul(p1[:], Wct[a][0][:, msl],
                                     zts[0][:, 0, :], start=True, stop=False)
                    nc.tensor.matmul(p1[:], Wct[a][1][:, msl],
                                     zts[0][:, 1, :], start=False, stop=False)
                    nc.tensor.matmul(p1[:], tcw[a][0][:, msl], onesrow[:],
                                     start=False, stop=True)
                    p2 = psmm.tile([P, T], f32, tag="mm",
                                   name=f"Hp2_{a}_{t}_{m}")
                    nc.tensor.matmul(p2[:], Wct[a][2][:, msl],
                                     zts[1][:, 0, :], start=True, stop=False)
                    nc.tensor.matmul(p2[:], Wct[a][3][:, msl],
                                     zts[1][:, 1, :], start=False, stop=False)
                    nc.tensor.matmul(p2[:], tcw[a][1][:, msl], onesrow[:],
                                     start=False, stop=True)
                    s1t = htm.tile([P, T], f32, tag="htm",
                                   name=f"Hs1_{a}_{t}_{m}")
                    nc.vector.tensor_tensor(s1t[:], p1[:], yb0[:], ALU.mult)
                    nc.vector.tensor_tensor(p2[:], p2[:], yb1[:], ALU.mult)
                    nc.vector.tensor_tensor(p2[:], p2[:], s1t[:], ALU.add)
                    nc.scalar.activation(xcz[2 * a + m][:, sl], p2[:],
                                         AF.Relu, bias=bct[a][m][:])
                    nc.vector.bn_stats(st_xcz[2 * a + m][:, t, :],
                                       xcz[2 * a + m][:, sl])

            # ---- output head for this attr (interleaved with combine) ----
            nk = HEAD_SIZES[a]
            stfh = stats_to_affine(f"arxc{a}", [st_xcz[2 * a],
                                                st_xcz[2 * a + 1]],
                                   [Gt["g1"][0], Gt["g1"][1]],
                                   [Gt["be1"][0], Gt["be1"][1]])
            bopp = fold_bias_col(f"fo{a}",
                                 lambda m, k, a=a: Wot[a][k][:],
                                 stfh, [bot[a]], 1, 2, mpart=nk)
            fold_weights(Wot[a], stfh)
            for t in range(NT):
                sl = slice(t * T, (t + 1) * T)
                ph = pss.tile([nk, T], f32, tag="pss", name=f"Ih{a}_{t}")
                for j in range(2):
                    nc.tensor.matmul(ph[:], Wot[a][j][:],
                                     xcz[2 * a + j][:, sl],
                                     start=(j == 0), stop=(j == 1))
                eh = htm.tile([nk, T], f32r, tag="htm", name=f"Ie{a}_{t}")
                nc.scalar.activation(eh[:], ph[:], AF.Exp, bias=bopp[0][:])
                psm = pss.tile([1, T], f32, tag="pss", name=f"Is{a}_{t}")
                nc.tensor.matmul(psm[:], onescol[nk][:], eh[:],
                                 start=True, stop=True)
                rin = htm.tile([1, T], f32, tag="htm2", name=f"Ir{a}_{t}")
                nc.vector.reciprocal_approx_fast(rin[:], psm[:])
                rb = ybp.tile([nk, T], f32, tag="yb", name=f"Irb{a}_{t}")
                nc.gpsimd.partition_broadcast(rb[:], rin[:])
                pro = htm.tile([nk, T], f32, tag="htm2", name=f"Ip{a}_{t}")
                nc.vector.tensor_tensor(pro[:], eh[:], rb[:], ALU.mult)
                nc.sync.dma_start(O[a].ap()[:, sl], pro[:])

        _stk.close()

    nc.compile()
    return nc


_CACHE = {}


def _get_nc(n_cores=N_CORES):
    if n_cores not in _CACHE:
        _CACHE[n_cores] = build(n_cores)
    return _CACHE[n_cores]


def make_in_maps(inputs, n_cores=N_CORES):
    f = lambda x: np.ascontiguousarray(np.asarray(x, dtype=np.float32))
    wy = f(inputs["Wy"])
    by = f(inputs["by"])
    shared = {
        "W1": f(inputs["W1"]), "b1": f(inputs["b1"]).reshape(256, 1),
        "W2": f(inputs["W2"]), "b2": f(inputs["b2"]).reshape(256, 1),
        "Wbx1": f(inputs["Wbx1"]), "bbx1": f(inputs["bbx1"]).reshape(4, 256, 1),
        "Wbx2": f(inputs["Wbx2"]), "bbx2": f(inputs["bbx2"]).reshape(4, 256, 1),
        "Wwa": f(inputs["Wwa"]), "bwa": f(inputs["bwa"]).reshape(4, 128, 1),
        "Wwb": f(inputs["Wwb"]), "bwb": f(inputs["bwb"]).reshape(4, 64, 1),
        "Wwc": f(inputs["Wwc"]), "bwc": f(inputs["bwc"]).reshape(4, 64, 1),
        "Wyd": np.ascontiguousarray((wy[:, :, 0] - wy[:, :, 1])[:, :, None]),
        "byd": np.ascontiguousarray((by[:, 0] - by[:, 1])[:, None, None]),
        "Wc": f(inputs["Wc"]), "bc": f(inputs["bc"]).reshape(4, 256, 1),
    }
    for k in range(4):
        n = HEAD_SIZES[k]
        shared[f"Wo{k+1}"] = f(inputs[f"Wo{k+1}".replace("Wo", "Wo")])
        shared[f"Wo{k+1}"] = f(inputs[f"Wo{k+1}"])
        shared[f"bo{k+1}"] = f(inputs[f"bo{k+1}"]).reshape(n, 1)
    for nm, d in [("g1", 256), ("be1", 256), ("g2", 256), ("be2", 256),
                  ("g3", 128), ("be3", 128), ("g4", 64), ("be4", 64)]:
        shared[nm] = f(inputs[nm]).reshape(d, 1)
    x1 = f(inputs["x1f"])
    x2 = f(inputs["x2f"])
    bl = x1.shape[0] // n_cores
    in_maps = []
    for c in range(n_cores):
        m = dict(shared)
        m["x1t"] = np.ascontiguousarray(x1[c * bl:(c + 1) * bl].T)
        m["x2t"] = np.ascontiguousarray(x2[c * bl:(c + 1) * bl].T)
        in_maps.append(m)
    return in_maps


def kernel(**inputs):
    nc = _get_nc(N_CORES)
    in_maps = make_in_maps(inputs, N_CORES)
    r = bass_utils.run_bass_kernel_spmd(nc, in_maps,
                                        core_ids=list(range(N_CORES)))
    outs = []
    for k in range(4):
        parts = [r.results[c][f"o{k+1}"].T for c in range(N_CORES)]
        outs.append(np.ascontiguousarray(np.concatenate(parts, axis=0)))
    return tuple(outs)
